# revision 23
# baseline (speedup 1.0000x reference)
"""Trainium2 Bass kernel for nn_DechunkModule (EMA dechunk/scan).

Computation (see reference):
  idx  = nonzero(boundary_mask)           # chunk boundary token ids
  p    = selected_probs[idx]              # [Lc]
  y_0  = concept_0 ; y_c = y_{c-1}*(1-p_c) + concept_c*p_c   (EMA over chunks)
  out[token] = y[chunk_of(token)]         # plug back, [1, L_TOK, HIDDEN]

Strategy: tensor-parallel over the hidden dim (8 cores x 512 columns).
Each core runs an identical Bass program on its hidden slice:

  - Chunks are tiled in blocks of T=96.  For each tile, the scan
    y = M' @ concept_tile + A_cum * carry  is ONE 97x97 @ 97x512 matmul:
    lhsT rows 0..95 hold the triangular coefficient matrix M'^T
    (M'[i,j] = p_j * prod_{k=j+1..i} a_k), row 96 holds the carry
    coefficients A_cum[i] = prod_{k=tile_start..i} a_k; the rhs holds the
    96 concept rows plus the running carry (previous tile's last row) in
    row 96.  Engine partition ranges must start at base 0/32/64/96, so
    both the carry row of the rhs and the carry OUTPUT row must sit at
    partition 96: each tile gets a 97th lhsT column that replicates the
    96th (via an extra scan column with a=1, D=0), so the matmul writes
    y[last chunk of tile] to PSUM row 96, which the next tile's rhs copy
    may legally read.
  - All tiles' coefficient matrices are built on-device by a single
    tensor_tensor_scan over a [97, 97*ntiles] layout (state = a*state+D):
    scan column 97t+i maps to chunk 96t+i for i<96 (column 97t+96 is the
    replica column).  The per-tile restart is encoded by zeroing `a` at
    tile starts, the diagonal injection D[j, 97t+j] = p_eff[96t+j]
    produces M'^T and the row-96 injection D[96, 97t] = a[96t] produces
    A_cum.
  - Chunk states are duplicated to their 2 tokens during the output DMA.
"""

import math

import numpy as np

import concourse.bacc as bacc
import concourse.mybir as mybir
import concourse.tile as tile
from concourse.bass_utils import run_bass_kernel_spmd

F32 = mybir.dt.float32
L_TOK = 16384
L_CHK = 8192
HIDDEN = 4096
NCORES = 8
T = 96  # chunks per tile (partition 96 of each matmul carries the running state)

_cache: dict = {}

# set by test harnesses to capture a hardware profile of the SPMD run
_PROFILE = False
_last_result = None


def _build(Lc: int, Hc: int, dup: int, use_f32r: bool = True):
    """Build the per-core Bass program for Lc chunks x Hc hidden columns.

    The output DRAM tensor is [dup * Lc, Hc]; chunk c is written to rows
    dup*c .. dup*c+dup-1 (token plug-back for uniform chunks of `dup` tokens).
    """
    ntiles = math.ceil(Lc / T)
    S = T + 1  # scan columns per tile (96 chunks + 1 replica column)
    Ls = S * ntiles
    nseg = 512  # a_bc build segment (PSUM bank width in fp32)

    F32R = mybir.dt.float32r
    MMDT = F32R if use_f32r else F32
    nc = bacc.Bacc("TRN2", target_bir_lowering=False, debug=False)
    concept_d = nc.dram_tensor("concept_s", [Lc, Hc], MMDT, kind="ExternalInput")
    amask_d = nc.dram_tensor("a_masked", [1, Ls], F32, kind="ExternalInput")
    arow_d = nc.dram_tensor("arow", [1, Ls], F32, kind="ExternalInput")
    pt_d = nc.dram_tensor("pT", [128, ntiles], F32, kind="ExternalInput")
    ident_d = nc.dram_tensor("ident", [128, 128], F32, kind="ExternalInput")
    ones_d = nc.dram_tensor("ones_r", [1, 128], F32, kind="ExternalInput")
    zeros_d = nc.dram_tensor("zeros_t", [T, Hc], MMDT, kind="ExternalInput")
    out_d = nc.dram_tensor("out", [dup * Lc, Hc], F32, kind="ExternalOutput")
    ov = out_d.rearrange("(c two) h -> c two h", two=dup)

    with tile.TileContext(nc) as tc:
        with (
            tc.tile_pool(name="const", bufs=1) as cpool,
            tc.tile_pool(name="rhs", bufs=4) as rhspool,
            tc.tile_pool(name="ysb", bufs=3) as ypool,
            tc.tile_pool(name="psum", bufs=8, space="PSUM") as psumpool,
        ):
            ident = cpool.tile([128, 128], F32)
            pt_sb = cpool.tile([128, ntiles], F32)
            mt = cpool.tile([128, Ls], MMDT, tag="mt")
            a_bc = cpool.tile([128, Ls], F32, tag="a_bc")
            dmat = cpool.tile([128, Ls], F32, tag="dmat")

            nc.sync.dma_start(ident[:], ident_d[:])
            nc.sync.dma_start(pt_sb[:], pt_d[:])
            # a_bc[p, c] = a_masked[c] for all p: partition-broadcast DMA read
            nc.sync.dma_start(
                a_bc[0:S, :], amask_d[0:1, :].to_broadcast((S, Ls))
            )
            # row 96 of D: carry-coefficient injections at tile starts
            nc.sync.dma_start(dmat[96:97, :], arow_d[:])

            # D rows 0..95: diagonal p_eff injections (identity * p column);
            # identity column 96 zeroes the replica column of each block.
            # On GpSimd so the D build overlaps the DVE scan below.
            for t in range(ntiles):
                nc.gpsimd.tensor_scalar(
                    dmat[0:T, S * t : S * t + S],
                    ident[0:T, 0:S],
                    pt_sb[0:T, t : t + 1],
                    None,
                    mybir.AluOpType.mult,
                )

            # the big scan: MT[:, c] = a_bc[:, c] * MT[:, c-1] + D[:, c]
            seg = 2048
            for s in range(0, Ls, seg):
                w = min(seg, Ls - s)
                init = 0.0 if s == 0 else mt[0:S, s - 1 : s]
                nc.vector.tensor_tensor_scan(
                    mt[0:S, s : s + w],
                    a_bc[0:S, s : s + w],
                    dmat[0:S, s : s + w],
                    init,
                    mybir.AluOpType.mult,
                    mybir.AluOpType.add,
                )

            # main pipeline over chunk tiles, in groups of G tiles: one
            # batched input DMA and one batched output DMA per group.  The
            # carry recurrence is split into NCH independent hidden-column
            # chains (chain h's carry copy runs on its own engine) so the
            # per-tile serial latency overlaps across chains.  ybig holds
            # each y row duplicated dup times so every SBUF partition row is
            # one contiguous (dup*Hc) run matching the contiguous DRAM
            # region: the DMA write packets become 4KB instead of 2KB.
            G = 4
            NCH = 2
            Hch = Hc // NCH
            def _copy(eng, out, in_):
                if eng is nc.scalar:
                    eng.copy(out, in_)
                else:
                    eng.tensor_copy(out, in_)

            carry_eng = [nc.scalar, nc.vector]
            nfull = Lc // T  # tiles with the full T chunks
            ngrp = nfull // G
            # grouped DRAM views: chunk row c = G*T*g + T*j + r
            if ngrp > 0:
                cv = concept_d[0 : ngrp * G * T, :].rearrange(
                    "(g j r) h -> g r j h", j=G, r=T
                )
                ovg = out_d[0 : dup * ngrp * G * T, :].rearrange(
                    "(g j r two) h -> g r j (two h)", j=G, r=T, two=dup
                )

            prev_ps = [None] * NCH
            for g in range(ngrp):
                btile = rhspool.tile([128, G * Hc], MMDT, tag="rhs")
                nc.sync.dma_start(
                    btile[0:T, :].rearrange("r (j h) -> r j h", j=G), cv[g]
                )
                ybig = ypool.tile([128, G * dup * Hc], F32, tag="ysb")
                for j in range(G):
                    t = G * g + j
                    for h in range(NCH):
                        cols = slice(j * Hc + h * Hch, j * Hc + (h + 1) * Hch)
                        if t == 0:
                            nc.sync.dma_start(
                                btile[T : T + 1, cols], zeros_d[0:1, 0:Hch]
                            )
                        else:
                            _copy(
                                carry_eng[h],
                                btile[T : T + 1, cols],
                                prev_ps[h][T : T + 1, :],
                            )
                        ps = psumpool.tile([128, Hch], F32, tag="ps")
                        nc.tensor.matmul(
                            ps[0 : T + 1, :],
                            mt[0:S, S * t : S * t + S],
                            btile[0:S, cols],
                            start=True, stop=True,
                        )
                        prev_ps[h] = ps
                        for r in range(dup):
                            ycols = slice(
                                (j * dup + r) * Hc + h * Hch,
                                (j * dup + r) * Hc + (h + 1) * Hch,
                            )
                            eng = nc.vector if (r + h + j) % 2 == 0 else nc.scalar
                            _copy(eng, ybig[0:T, ycols], ps[0:T, :])
                nc.sync.dma_start(
                    ovg[g],
                    ybig[0:T, :].rearrange("r (j x) -> r j x", j=G),
                )

            # leftover tiles (tail that doesn't fill a group)
            ovt = out_d.rearrange("(c two) h -> c (two h)", two=dup)
            for t in range(G * ngrp, ntiles):
                c0 = T * t
                w = min(T, Lc - c0)
                m = w + 1 if w == T else w  # full tiles emit the carry row
                rhs = rhspool.tile([128, G * Hc], MMDT, tag="rhs")
                if w < T:
                    # partial tile: zero the pad rows (their lhsT columns
                    # are all-zero, but operands must be initialized)
                    nc.sync.dma_start(rhs[0:T, 0:Hc], zeros_d[:])
                nc.sync.dma_start(rhs[0:w, 0:Hc], concept_d[c0 : c0 + w, :])
                y_sb = ypool.tile([128, G * dup * Hc], F32, tag="ysb")
                for h in range(NCH):
                    cols = slice(h * Hch, (h + 1) * Hch)
                    if t == 0:
                        nc.sync.dma_start(
                            rhs[T : T + 1, cols], zeros_d[0:1, 0:Hch]
                        )
                    else:
                        _copy(
                            carry_eng[h],
                            rhs[T : T + 1, cols],
                            prev_ps[h][T : T + 1, :],
                        )
                    ps = psumpool.tile([128, Hch], F32, tag="ps")
                    nc.tensor.matmul(
                        ps[0:m, :],
                        mt[0:S, S * t : S * t + m],
                        rhs[0:S, cols],
                        start=True, stop=True,
                    )
                    prev_ps[h] = ps
                    for r in range(dup):
                        ycols = slice(r * Hc + h * Hch, r * Hc + (h + 1) * Hch)
                        eng = nc.vector if (r + h) % 2 == 0 else nc.scalar
                        _copy(eng, y_sb[0:w, ycols], ps[0:w, :])
                nc.sync.dma_start(ovt[c0 : c0 + w, :], y_sb[0:w, 0 : dup * Hc])

    nc.compile()
    return nc


def _get_program(Lc: int, Hc: int, dup: int):
    key = (Lc, Hc, dup)
    if key not in _cache:
        _cache[key] = _build(Lc, Hc, dup)
    return _cache[key]


def _host_prep(concept, selected_probs, boundary_mask):
    """Derive the per-chunk scan coefficients (replicated across cores)."""
    Lc = concept.shape[1]
    mask = np.asarray(boundary_mask[0])
    probs = np.asarray(selected_probs[0, :, 0], dtype=np.float32)

    idx = np.nonzero(mask)[0]
    if len(idx) < Lc:
        idx = np.pad(idx, (0, Lc - len(idx)))
    idx = idx[:Lc]
    p = probs[idx].astype(np.float32)
    a = (1.0 - p).astype(np.float32)
    a[0] = 0.0
    p_eff = p.copy()
    p_eff[0] = 1.0

    ntiles = math.ceil(Lc / T)
    S = T + 1
    Ls = S * ntiles
    # scan-space coefficients: column S*t+i <-> chunk T*t+i (i < T);
    # column S*t+T is the replica column (a=1, injections=0)
    a_scan = np.zeros(Ls, np.float32)
    arow = np.zeros(Ls, np.float32)
    pt = np.zeros((128, ntiles), np.float32)
    for t in range(ntiles):
        w = min(T, Lc - T * t)
        a_scan[S * t : S * t + w] = a[T * t : T * t + w]
        a_scan[S * t] = 0.0
        a_scan[S * t + T] = 1.0
        arow[S * t] = a[T * t]
        pt[:w, t] = p_eff[T * t : T * t + w]
    return a_scan[None, :], arow[None, :], pt, mask, idx


def kernel(concept, selected_probs, boundary_mask):
    concept = np.asarray(concept, dtype=np.float32)
    selected_probs = np.asarray(selected_probs, dtype=np.float32)
    boundary_mask = np.asarray(boundary_mask)

    Lc = concept.shape[1]
    L = boundary_mask.shape[1]
    H = concept.shape[2]
    Hc = H // NCORES

    a_masked, arow, pt, mask, idx = _host_prep(
        concept, selected_probs, boundary_mask
    )

    # fast path: uniform chunks of `dup` tokens (the setup_inputs pattern)
    dup = L // Lc
    expect = np.zeros(L, dtype=bool)
    expect[0::dup] = True
    uniform = dup * Lc == L and bool(np.array_equal(mask, expect))

    nc = _get_program(Lc, Hc, dup if uniform else 1)

    ident = np.eye(128, dtype=np.float32)
    ones_r = np.ones((1, 128), np.float32)
    zeros_t = np.zeros((T, Hc), np.float32)
    in_maps = []
    for k in range(NCORES):
        in_maps.append(
            {
                "concept_s": np.ascontiguousarray(
                    concept[0, :, k * Hc : (k + 1) * Hc]
                ),
                "a_masked": a_masked,
                "arow": arow,
                "pT": pt,
                "ident": ident,
                "ones_r": ones_r,
                "zeros_t": zeros_t,
            }
        )

    res = run_bass_kernel_spmd(
        nc, in_maps, list(range(NCORES)), trace=_PROFILE
    )
    global _last_result
    _last_result = res

    rows = dup * Lc if uniform else Lc
    full = np.empty((rows, H), np.float32)
    for k in range(NCORES):
        full[:, k * Hc : (k + 1) * Hc] = res.results[k]["out"]

    if uniform:
        return full[None]

    # general fallback: device computed the chunk states; plug back on host
    merge = full
    plug = np.cumsum(mask.astype(np.int64)) - 1
    plug = np.clip(plug, 0, Lc - 1)
    return merge[plug][None]


# revision 25
# speedup vs baseline: 1.0082x; 1.0082x over previous
"""Trainium2 Bass kernel for nn_DechunkModule (EMA dechunk/scan).

Computation (see reference):
  idx  = nonzero(boundary_mask)           # chunk boundary token ids
  p    = selected_probs[idx]              # [Lc]
  y_0  = concept_0 ; y_c = y_{c-1}*(1-p_c) + concept_c*p_c   (EMA over chunks)
  out[token] = y[chunk_of(token)]         # plug back, [1, L_TOK, HIDDEN]

Strategy: tensor-parallel over the hidden dim (8 cores x 512 columns).
Each core runs an identical Bass program on its hidden slice:

  - Chunks are tiled in blocks of T=96.  For each tile, the scan
    y = M' @ concept_tile + A_cum * carry  is ONE 97x97 @ 97x512 matmul:
    lhsT rows 0..95 hold the triangular coefficient matrix M'^T
    (M'[i,j] = p_j * prod_{k=j+1..i} a_k), row 96 holds the carry
    coefficients A_cum[i] = prod_{k=tile_start..i} a_k; the rhs holds the
    96 concept rows plus the running carry (previous tile's last row) in
    row 96.  Engine partition ranges must start at base 0/32/64/96, so
    both the carry row of the rhs and the carry OUTPUT row must sit at
    partition 96: each tile gets a 97th lhsT column that replicates the
    96th (via an extra scan column with a=1, D=0), so the matmul writes
    y[last chunk of tile] to PSUM row 96, which the next tile's rhs copy
    may legally read.
  - All tiles' coefficient matrices are built on-device by a single
    tensor_tensor_scan over a [97, 97*ntiles] layout (state = a*state+D):
    scan column 97t+i maps to chunk 96t+i for i<96 (column 97t+96 is the
    replica column).  The per-tile restart is encoded by zeroing `a` at
    tile starts, the diagonal injection D[j, 97t+j] = p_eff[96t+j]
    produces M'^T and the row-96 injection D[96, 97t] = a[96t] produces
    A_cum.
  - Chunk states are duplicated to their 2 tokens during the output DMA.
"""

import math

import numpy as np

import concourse.bacc as bacc
import concourse.mybir as mybir
import concourse.tile as tile
from concourse.bass_utils import run_bass_kernel_spmd

F32 = mybir.dt.float32
L_TOK = 16384
L_CHK = 8192
HIDDEN = 4096
NCORES = 8
T = 96  # chunks per tile (partition 96 of each matmul carries the running state)

_cache: dict = {}

# set by test harnesses to capture a hardware profile of the SPMD run
_PROFILE = False
_last_result = None


def _build(Lc: int, Hc: int, dup: int, use_f32r: bool = True):
    """Build the per-core Bass program for Lc chunks x Hc hidden columns.

    The output DRAM tensor is [dup * Lc, Hc]; chunk c is written to rows
    dup*c .. dup*c+dup-1 (token plug-back for uniform chunks of `dup` tokens).
    """
    ntiles = math.ceil(Lc / T)
    S = T + 1  # scan columns per tile (96 chunks + 1 replica column)
    Ls = S * ntiles
    nseg = 512  # a_bc build segment (PSUM bank width in fp32)

    F32R = mybir.dt.float32r
    MMDT = F32R if use_f32r else F32
    nc = bacc.Bacc("TRN2", target_bir_lowering=False, debug=False)
    concept_d = nc.dram_tensor("concept_s", [Lc, Hc], MMDT, kind="ExternalInput")
    amask_d = nc.dram_tensor("a_masked", [S, Ls], F32, kind="ExternalInput")
    arow_d = nc.dram_tensor("arow", [1, Ls], F32, kind="ExternalInput")
    pt_d = nc.dram_tensor("pT", [128, ntiles], F32, kind="ExternalInput")
    ident_d = nc.dram_tensor("ident", [128, 128], F32, kind="ExternalInput")
    ones_d = nc.dram_tensor("ones_r", [1, 128], F32, kind="ExternalInput")
    zeros_d = nc.dram_tensor("zeros_t", [T, Hc], MMDT, kind="ExternalInput")
    out_d = nc.dram_tensor("out", [dup * Lc, Hc], F32, kind="ExternalOutput")
    ov = out_d.rearrange("(c two) h -> c two h", two=dup)

    with tile.TileContext(nc) as tc:
        with (
            tc.tile_pool(name="const", bufs=1) as cpool,
            tc.tile_pool(name="rhs", bufs=4) as rhspool,
            tc.tile_pool(name="ysb", bufs=3) as ypool,
            tc.tile_pool(name="psum", bufs=8, space="PSUM") as psumpool,
        ):
            ident = cpool.tile([128, 128], F32)
            pt_sb = cpool.tile([128, ntiles], F32)
            mt = cpool.tile([128, Ls], MMDT, tag="mt")
            a_bc = cpool.tile([128, Ls], F32, tag="a_bc")
            dmat = cpool.tile([128, Ls], F32, tag="dmat")

            nc.sync.dma_start(ident[:], ident_d[:])
            nc.sync.dma_start(pt_sb[:], pt_d[:])
            # a_bc[p, c] = a_masked[c] for all p (pre-broadcast on host)
            nc.sync.dma_start(a_bc[0:S, :], amask_d[:])
            # row 96 of D: carry-coefficient injections at tile starts
            nc.sync.dma_start(dmat[96:97, :], arow_d[:])

            # D rows 0..95: diagonal p_eff injections (identity * p column);
            # identity column 96 zeroes the replica column of each block.
            for t in range(ntiles):
                nc.vector.tensor_scalar(
                    dmat[0:T, S * t : S * t + S],
                    ident[0:T, 0:S],
                    pt_sb[0:T, t : t + 1],
                    None,
                    mybir.AluOpType.mult,
                )

            # the big scan: MT[:, c] = a_bc[:, c] * MT[:, c-1] + D[:, c]
            seg = 2048
            for s in range(0, Ls, seg):
                w = min(seg, Ls - s)
                init = 0.0 if s == 0 else mt[0:S, s - 1 : s]
                nc.vector.tensor_tensor_scan(
                    mt[0:S, s : s + w],
                    a_bc[0:S, s : s + w],
                    dmat[0:S, s : s + w],
                    init,
                    mybir.AluOpType.mult,
                    mybir.AluOpType.add,
                )

            # main pipeline over chunk tiles, in groups of G tiles: one
            # batched input DMA and one batched output DMA per group.  The
            # carry recurrence is split into NCH independent hidden-column
            # chains (chain h's carry copy runs on its own engine) so the
            # per-tile serial latency overlaps across chains.  ybig holds
            # each y row duplicated dup times so every SBUF partition row is
            # one contiguous (dup*Hc) run matching the contiguous DRAM
            # region: the DMA write packets become 4KB instead of 2KB.
            G = 4
            NCH = 2
            Hch = Hc // NCH
            def _copy(eng, out, in_):
                if eng is nc.scalar:
                    eng.copy(out, in_)
                else:
                    eng.tensor_copy(out, in_)

            carry_eng = [nc.scalar, nc.vector]
            nfull = Lc // T  # tiles with the full T chunks
            ngrp = nfull // G
            # grouped DRAM views: chunk row c = G*T*g + T*j + r
            if ngrp > 0:
                cv = concept_d[0 : ngrp * G * T, :].rearrange(
                    "(g j r) h -> g r j h", j=G, r=T
                )
                ovg = out_d[0 : dup * ngrp * G * T, :].rearrange(
                    "(g j r two) h -> g r j (two h)", j=G, r=T, two=dup
                )

            prev_ps = [None] * NCH
            for g in range(ngrp):
                btile = rhspool.tile([128, G * Hc], MMDT, tag="rhs")
                nc.sync.dma_start(
                    btile[0:T, :].rearrange("r (j h) -> r j h", j=G), cv[g]
                )
                ybig = ypool.tile([128, G * dup * Hc], F32, tag="ysb")
                for j in range(G):
                    t = G * g + j
                    for h in range(NCH):
                        cols = slice(j * Hc + h * Hch, j * Hc + (h + 1) * Hch)
                        if t == 0:
                            nc.sync.dma_start(
                                btile[T : T + 1, cols], zeros_d[0:1, 0:Hch]
                            )
                        else:
                            _copy(
                                carry_eng[h],
                                btile[T : T + 1, cols],
                                prev_ps[h][T : T + 1, :],
                            )
                        ps = psumpool.tile([128, Hch], F32, tag="ps")
                        nc.tensor.matmul(
                            ps[0 : T + 1, :],
                            mt[0:S, S * t : S * t + S],
                            btile[0:S, cols],
                            start=True, stop=True,
                        )
                        prev_ps[h] = ps
                        for r in range(dup):
                            ycols = slice(
                                (j * dup + r) * Hc + h * Hch,
                                (j * dup + r) * Hc + (h + 1) * Hch,
                            )
                            eng = nc.vector if (r + h + j) % 2 == 0 else nc.scalar
                            _copy(eng, ybig[0:T, ycols], ps[0:T, :])
                nc.sync.dma_start(
                    ovg[g],
                    ybig[0:T, :].rearrange("r (j x) -> r j x", j=G),
                )

            # leftover tiles (tail that doesn't fill a group)
            ovt = out_d.rearrange("(c two) h -> c (two h)", two=dup)
            for t in range(G * ngrp, ntiles):
                c0 = T * t
                w = min(T, Lc - c0)
                m = w + 1 if w == T else w  # full tiles emit the carry row
                rhs = rhspool.tile([128, G * Hc], MMDT, tag="rhs")
                if w < T:
                    # partial tile: zero the pad rows (their lhsT columns
                    # are all-zero, but operands must be initialized)
                    nc.sync.dma_start(rhs[0:T, 0:Hc], zeros_d[:])
                nc.sync.dma_start(rhs[0:w, 0:Hc], concept_d[c0 : c0 + w, :])
                y_sb = ypool.tile([128, G * dup * Hc], F32, tag="ysb")
                for h in range(NCH):
                    cols = slice(h * Hch, (h + 1) * Hch)
                    if t == 0:
                        nc.sync.dma_start(
                            rhs[T : T + 1, cols], zeros_d[0:1, 0:Hch]
                        )
                    else:
                        _copy(
                            carry_eng[h],
                            rhs[T : T + 1, cols],
                            prev_ps[h][T : T + 1, :],
                        )
                    ps = psumpool.tile([128, Hch], F32, tag="ps")
                    nc.tensor.matmul(
                        ps[0:m, :],
                        mt[0:S, S * t : S * t + m],
                        rhs[0:S, cols],
                        start=True, stop=True,
                    )
                    prev_ps[h] = ps
                    for r in range(dup):
                        ycols = slice(r * Hc + h * Hch, r * Hc + (h + 1) * Hch)
                        eng = nc.vector if (r + h) % 2 == 0 else nc.scalar
                        _copy(eng, y_sb[0:w, ycols], ps[0:w, :])
                nc.sync.dma_start(ovt[c0 : c0 + w, :], y_sb[0:w, 0 : dup * Hc])

    nc.compile()
    return nc


def _get_program(Lc: int, Hc: int, dup: int):
    key = (Lc, Hc, dup)
    if key not in _cache:
        _cache[key] = _build(Lc, Hc, dup)
    return _cache[key]


def _host_prep(concept, selected_probs, boundary_mask):
    """Derive the per-chunk scan coefficients (replicated across cores)."""
    Lc = concept.shape[1]
    mask = np.asarray(boundary_mask[0])
    probs = np.asarray(selected_probs[0, :, 0], dtype=np.float32)

    idx = np.nonzero(mask)[0]
    if len(idx) < Lc:
        idx = np.pad(idx, (0, Lc - len(idx)))
    idx = idx[:Lc]
    p = probs[idx].astype(np.float32)
    a = (1.0 - p).astype(np.float32)
    a[0] = 0.0
    p_eff = p.copy()
    p_eff[0] = 1.0

    ntiles = math.ceil(Lc / T)
    S = T + 1
    Ls = S * ntiles
    # scan-space coefficients: column S*t+i <-> chunk T*t+i (i < T);
    # column S*t+T is the replica column (a=1, injections=0)
    a_scan = np.zeros(Ls, np.float32)
    arow = np.zeros(Ls, np.float32)
    pt = np.zeros((128, ntiles), np.float32)
    for t in range(ntiles):
        w = min(T, Lc - T * t)
        a_scan[S * t : S * t + w] = a[T * t : T * t + w]
        a_scan[S * t] = 0.0
        a_scan[S * t + T] = 1.0
        arow[S * t] = a[T * t]
        pt[:w, t] = p_eff[T * t : T * t + w]
    a_bcast = np.ascontiguousarray(
        np.broadcast_to(a_scan, (T + 2, Ls))[: T + 1]
    )
    return a_bcast, arow[None, :], pt, mask, idx


def kernel(concept, selected_probs, boundary_mask):
    concept = np.asarray(concept, dtype=np.float32)
    selected_probs = np.asarray(selected_probs, dtype=np.float32)
    boundary_mask = np.asarray(boundary_mask)

    Lc = concept.shape[1]
    L = boundary_mask.shape[1]
    H = concept.shape[2]
    Hc = H // NCORES

    a_masked, arow, pt, mask, idx = _host_prep(
        concept, selected_probs, boundary_mask
    )

    # fast path: uniform chunks of `dup` tokens (the setup_inputs pattern)
    dup = L // Lc
    expect = np.zeros(L, dtype=bool)
    expect[0::dup] = True
    uniform = dup * Lc == L and bool(np.array_equal(mask, expect))

    nc = _get_program(Lc, Hc, dup if uniform else 1)

    ident = np.eye(128, dtype=np.float32)
    ones_r = np.ones((1, 128), np.float32)
    zeros_t = np.zeros((T, Hc), np.float32)
    in_maps = []
    for k in range(NCORES):
        in_maps.append(
            {
                "concept_s": np.ascontiguousarray(
                    concept[0, :, k * Hc : (k + 1) * Hc]
                ),
                "a_masked": a_masked,
                "arow": arow,
                "pT": pt,
                "ident": ident,
                "ones_r": ones_r,
                "zeros_t": zeros_t,
            }
        )

    res = run_bass_kernel_spmd(
        nc, in_maps, list(range(NCORES)), trace=_PROFILE
    )
    global _last_result
    _last_result = res

    rows = dup * Lc if uniform else Lc
    full = np.empty((rows, H), np.float32)
    for k in range(NCORES):
        full[:, k * Hc : (k + 1) * Hc] = res.results[k]["out"]

    if uniform:
        return full[None]

    # general fallback: device computed the chunk states; plug back on host
    merge = full
    plug = np.cumsum(mask.astype(np.int64)) - 1
    plug = np.clip(plug, 0, Lc - 1)
    return merge[plug][None]


# revision 26
# speedup vs baseline: 1.0788x; 1.0700x over previous
"""Trainium2 Bass kernel for nn_DechunkModule (EMA dechunk/scan).

Computation (see reference):
  idx  = nonzero(boundary_mask)           # chunk boundary token ids
  p    = selected_probs[idx]              # [Lc]
  y_0  = concept_0 ; y_c = y_{c-1}*(1-p_c) + concept_c*p_c   (EMA over chunks)
  out[token] = y[chunk_of(token)]         # plug back, [1, L_TOK, HIDDEN]

Strategy: tensor-parallel over the hidden dim (8 cores x 512 columns).
Each core runs an identical Bass program on its hidden slice:

  - Chunks are tiled in blocks of T=96.  For each tile, the scan
    y = M' @ concept_tile + A_cum * carry  is ONE 97x97 @ 97x512 matmul:
    lhsT rows 0..95 hold the triangular coefficient matrix M'^T
    (M'[i,j] = p_j * prod_{k=j+1..i} a_k), row 96 holds the carry
    coefficients A_cum[i] = prod_{k=tile_start..i} a_k; the rhs holds the
    96 concept rows plus the running carry (previous tile's last row) in
    row 96.  Engine partition ranges must start at base 0/32/64/96, so
    both the carry row of the rhs and the carry OUTPUT row must sit at
    partition 96: each tile gets a 97th lhsT column that replicates the
    96th (via an extra scan column with a=1, D=0), so the matmul writes
    y[last chunk of tile] to PSUM row 96, which the next tile's rhs copy
    may legally read.
  - All tiles' coefficient matrices are built on-device by a single
    tensor_tensor_scan over a [97, 97*ntiles] layout (state = a*state+D):
    scan column 97t+i maps to chunk 96t+i for i<96 (column 97t+96 is the
    replica column).  The per-tile restart is encoded by zeroing `a` at
    tile starts, the diagonal injection D[j, 97t+j] = p_eff[96t+j]
    produces M'^T and the row-96 injection D[96, 97t] = a[96t] produces
    A_cum.
  - Chunk states are duplicated to their 2 tokens during the output DMA.
"""

import math

import numpy as np

import concourse.bacc as bacc
import concourse.mybir as mybir
import concourse.tile as tile
from concourse.bass_utils import run_bass_kernel_spmd

F32 = mybir.dt.float32
L_TOK = 16384
L_CHK = 8192
HIDDEN = 4096
NCORES = 8
T = 96  # chunks per tile (partition 96 of each matmul carries the running state)

_cache: dict = {}

# set by test harnesses to capture a hardware profile of the SPMD run
_PROFILE = False
_last_result = None


def _build(Lc: int, Hc: int, dup: int, use_f32r: bool = True):
    """Build the per-core Bass program for Lc chunks x Hc hidden columns.

    The output DRAM tensor is [dup * Lc, Hc]; chunk c is written to rows
    dup*c .. dup*c+dup-1 (token plug-back for uniform chunks of `dup` tokens).
    """
    ntiles = math.ceil(Lc / T)
    S = T + 1  # scan columns per tile (96 chunks + 1 replica column)
    Ls = S * ntiles
    nseg = 512  # a_bc build segment (PSUM bank width in fp32)

    F32R = mybir.dt.float32r
    MMDT = F32R if use_f32r else F32
    nc = bacc.Bacc("TRN2", target_bir_lowering=False, debug=False)
    concept_d = nc.dram_tensor("concept_s", [Lc, Hc], MMDT, kind="ExternalInput")
    amask_d = nc.dram_tensor("a_masked", [S, Ls], F32, kind="ExternalInput")
    arow_d = nc.dram_tensor("arow", [1, Ls], F32, kind="ExternalInput")
    pt_d = nc.dram_tensor("pT", [128, ntiles], F32, kind="ExternalInput")
    ident_d = nc.dram_tensor("ident", [128, 128], F32, kind="ExternalInput")
    ones_d = nc.dram_tensor("ones_r", [1, 128], F32, kind="ExternalInput")
    zeros_d = nc.dram_tensor("zeros_t", [T, Hc], MMDT, kind="ExternalInput")
    out_d = nc.dram_tensor("out", [dup * Lc, Hc], F32, kind="ExternalOutput")
    ov = out_d.rearrange("(c two) h -> c two h", two=dup)

    with tile.TileContext(nc) as tc:
        with (
            tc.tile_pool(name="const", bufs=1) as cpool,
            tc.tile_pool(name="rhs", bufs=4) as rhspool,
            tc.tile_pool(name="ysb", bufs=3) as ypool,
            tc.tile_pool(name="psum", bufs=8, space="PSUM") as psumpool,
        ):
            ident = cpool.tile([128, 128], F32)
            pt_sb = cpool.tile([128, ntiles], F32)
            mt = cpool.tile([128, Ls], MMDT, tag="mt")
            a_bc = cpool.tile([128, Ls], F32, tag="a_bc")
            dmat = cpool.tile([128, Ls], F32, tag="dmat")

            nc.sync.dma_start(ident[:], ident_d[:])
            nc.sync.dma_start(pt_sb[:], pt_d[:])
            # a_bc[p, c] = a_masked[c] for all p (pre-broadcast on host);
            # segmented on the ACT ring so the scan can start after seg 0
            for s in range(0, Ls, 2048):
                w = min(2048, Ls - s)
                nc.scalar.dma_start(a_bc[0:S, s : s + w], amask_d[:, s : s + w])
            # row 96 of D: carry-coefficient injections at tile starts
            nc.sync.dma_start(dmat[96:97, :], arow_d[:])

            # D rows 0..95: diagonal p_eff injections (identity * p column);
            # identity column 96 zeroes the replica column of each block.
            for t in range(ntiles):
                nc.vector.tensor_scalar(
                    dmat[0:T, S * t : S * t + S],
                    ident[0:T, 0:S],
                    pt_sb[0:T, t : t + 1],
                    None,
                    mybir.AluOpType.mult,
                )

            # the big scan: MT[:, c] = a_bc[:, c] * MT[:, c-1] + D[:, c]
            seg = 2048
            for s in range(0, Ls, seg):
                w = min(seg, Ls - s)
                init = 0.0 if s == 0 else mt[0:S, s - 1 : s]
                nc.vector.tensor_tensor_scan(
                    mt[0:S, s : s + w],
                    a_bc[0:S, s : s + w],
                    dmat[0:S, s : s + w],
                    init,
                    mybir.AluOpType.mult,
                    mybir.AluOpType.add,
                )

            # main pipeline over chunk tiles, in groups of G tiles: one
            # batched input DMA and one batched output DMA per group.  The
            # carry recurrence is split into NCH independent hidden-column
            # chains (chain h's carry copy runs on its own engine) so the
            # per-tile serial latency overlaps across chains.  ybig holds
            # each y row duplicated dup times so every SBUF partition row is
            # one contiguous (dup*Hc) run matching the contiguous DRAM
            # region: the DMA write packets become 4KB instead of 2KB.
            G = 4
            NCH = 2
            Hch = Hc // NCH
            def _copy(eng, out, in_):
                if eng is nc.scalar:
                    eng.copy(out, in_)
                else:
                    eng.tensor_copy(out, in_)

            carry_eng = [nc.scalar, nc.vector]
            nfull = Lc // T  # tiles with the full T chunks
            ngrp = nfull // G
            # grouped DRAM views: chunk row c = G*T*g + T*j + r
            if ngrp > 0:
                cv = concept_d[0 : ngrp * G * T, :].rearrange(
                    "(g j r) h -> g r j h", j=G, r=T
                )
                ovg = out_d[0 : dup * ngrp * G * T, :].rearrange(
                    "(g j r two) h -> g r j (two h)", j=G, r=T, two=dup
                )

            prev_ps = [None] * NCH
            for g in range(ngrp):
                btile = rhspool.tile([128, G * Hc], MMDT, tag="rhs")
                nc.scalar.dma_start(
                    btile[0:T, :].rearrange("r (j h) -> r j h", j=G), cv[g]
                )
                ybig = ypool.tile([128, G * dup * Hc], F32, tag="ysb")
                for j in range(G):
                    t = G * g + j
                    for h in range(NCH):
                        cols = slice(j * Hc + h * Hch, j * Hc + (h + 1) * Hch)
                        if t == 0:
                            nc.sync.dma_start(
                                btile[T : T + 1, cols], zeros_d[0:1, 0:Hch]
                            )
                        else:
                            _copy(
                                carry_eng[h],
                                btile[T : T + 1, cols],
                                prev_ps[h][T : T + 1, :],
                            )
                        ps = psumpool.tile([128, Hch], F32, tag="ps")
                        nc.tensor.matmul(
                            ps[0 : T + 1, :],
                            mt[0:S, S * t : S * t + S],
                            btile[0:S, cols],
                            start=True, stop=True,
                        )
                        prev_ps[h] = ps
                        for r in range(dup):
                            ycols = slice(
                                (j * dup + r) * Hc + h * Hch,
                                (j * dup + r) * Hc + (h + 1) * Hch,
                            )
                            eng = nc.vector if (r + h + j) % 2 == 0 else nc.scalar
                            _copy(eng, ybig[0:T, ycols], ps[0:T, :])
                nc.sync.dma_start(
                    ovg[g],
                    ybig[0:T, :].rearrange("r (j x) -> r j x", j=G),
                )

            # leftover tiles (tail that doesn't fill a group)
            ovt = out_d.rearrange("(c two) h -> c (two h)", two=dup)
            for t in range(G * ngrp, ntiles):
                c0 = T * t
                w = min(T, Lc - c0)
                m = w + 1 if w == T else w  # full tiles emit the carry row
                rhs = rhspool.tile([128, G * Hc], MMDT, tag="rhs")
                if w < T:
                    # partial tile: zero the pad rows (their lhsT columns
                    # are all-zero, but operands must be initialized)
                    nc.sync.dma_start(rhs[0:T, 0:Hc], zeros_d[:])
                nc.scalar.dma_start(rhs[0:w, 0:Hc], concept_d[c0 : c0 + w, :])
                y_sb = ypool.tile([128, G * dup * Hc], F32, tag="ysb")
                for h in range(NCH):
                    cols = slice(h * Hch, (h + 1) * Hch)
                    if t == 0:
                        nc.sync.dma_start(
                            rhs[T : T + 1, cols], zeros_d[0:1, 0:Hch]
                        )
                    else:
                        _copy(
                            carry_eng[h],
                            rhs[T : T + 1, cols],
                            prev_ps[h][T : T + 1, :],
                        )
                    ps = psumpool.tile([128, Hch], F32, tag="ps")
                    nc.tensor.matmul(
                        ps[0:m, :],
                        mt[0:S, S * t : S * t + m],
                        rhs[0:S, cols],
                        start=True, stop=True,
                    )
                    prev_ps[h] = ps
                    for r in range(dup):
                        ycols = slice(r * Hc + h * Hch, r * Hc + (h + 1) * Hch)
                        eng = nc.vector if (r + h) % 2 == 0 else nc.scalar
                        _copy(eng, y_sb[0:w, ycols], ps[0:w, :])
                nc.sync.dma_start(ovt[c0 : c0 + w, :], y_sb[0:w, 0 : dup * Hc])

    nc.compile()
    return nc


def _get_program(Lc: int, Hc: int, dup: int):
    key = (Lc, Hc, dup)
    if key not in _cache:
        _cache[key] = _build(Lc, Hc, dup)
    return _cache[key]


def _host_prep(concept, selected_probs, boundary_mask):
    """Derive the per-chunk scan coefficients (replicated across cores)."""
    Lc = concept.shape[1]
    mask = np.asarray(boundary_mask[0])
    probs = np.asarray(selected_probs[0, :, 0], dtype=np.float32)

    idx = np.nonzero(mask)[0]
    if len(idx) < Lc:
        idx = np.pad(idx, (0, Lc - len(idx)))
    idx = idx[:Lc]
    p = probs[idx].astype(np.float32)
    a = (1.0 - p).astype(np.float32)
    a[0] = 0.0
    p_eff = p.copy()
    p_eff[0] = 1.0

    ntiles = math.ceil(Lc / T)
    S = T + 1
    Ls = S * ntiles
    # scan-space coefficients: column S*t+i <-> chunk T*t+i (i < T);
    # column S*t+T is the replica column (a=1, injections=0)
    a_scan = np.zeros(Ls, np.float32)
    arow = np.zeros(Ls, np.float32)
    pt = np.zeros((128, ntiles), np.float32)
    for t in range(ntiles):
        w = min(T, Lc - T * t)
        a_scan[S * t : S * t + w] = a[T * t : T * t + w]
        a_scan[S * t] = 0.0
        a_scan[S * t + T] = 1.0
        arow[S * t] = a[T * t]
        pt[:w, t] = p_eff[T * t : T * t + w]
    a_bcast = np.ascontiguousarray(
        np.broadcast_to(a_scan, (T + 2, Ls))[: T + 1]
    )
    return a_bcast, arow[None, :], pt, mask, idx


def kernel(concept, selected_probs, boundary_mask):
    concept = np.asarray(concept, dtype=np.float32)
    selected_probs = np.asarray(selected_probs, dtype=np.float32)
    boundary_mask = np.asarray(boundary_mask)

    Lc = concept.shape[1]
    L = boundary_mask.shape[1]
    H = concept.shape[2]
    Hc = H // NCORES

    a_masked, arow, pt, mask, idx = _host_prep(
        concept, selected_probs, boundary_mask
    )

    # fast path: uniform chunks of `dup` tokens (the setup_inputs pattern)
    dup = L // Lc
    expect = np.zeros(L, dtype=bool)
    expect[0::dup] = True
    uniform = dup * Lc == L and bool(np.array_equal(mask, expect))

    nc = _get_program(Lc, Hc, dup if uniform else 1)

    ident = np.eye(128, dtype=np.float32)
    ones_r = np.ones((1, 128), np.float32)
    zeros_t = np.zeros((T, Hc), np.float32)
    in_maps = []
    for k in range(NCORES):
        in_maps.append(
            {
                "concept_s": np.ascontiguousarray(
                    concept[0, :, k * Hc : (k + 1) * Hc]
                ),
                "a_masked": a_masked,
                "arow": arow,
                "pT": pt,
                "ident": ident,
                "ones_r": ones_r,
                "zeros_t": zeros_t,
            }
        )

    res = run_bass_kernel_spmd(
        nc, in_maps, list(range(NCORES)), trace=_PROFILE
    )
    global _last_result
    _last_result = res

    rows = dup * Lc if uniform else Lc
    full = np.empty((rows, H), np.float32)
    for k in range(NCORES):
        full[:, k * Hc : (k + 1) * Hc] = res.results[k]["out"]

    if uniform:
        return full[None]

    # general fallback: device computed the chunk states; plug back on host
    merge = full
    plug = np.cumsum(mask.astype(np.int64)) - 1
    plug = np.clip(plug, 0, Lc - 1)
    return merge[plug][None]


# revision 29
# speedup vs baseline: 1.4600x; 1.3533x over previous
"""Trainium2 Bass kernel for nn_DechunkModule (EMA dechunk/scan).

Computation (see reference):
  idx  = nonzero(boundary_mask)           # chunk boundary token ids
  p    = selected_probs[idx]              # [Lc]
  y_0  = concept_0 ; y_c = y_{c-1}*(1-p_c) + concept_c*p_c   (EMA over chunks)
  out[token] = y[chunk_of(token)]         # plug back, [1, L_TOK, HIDDEN]

Strategy: tensor-parallel over the hidden dim (8 cores x 512 columns).
Each core runs an identical Bass program on its hidden slice:

  - Chunks are tiled in blocks of T=96.  For each tile, the scan
    y = M' @ concept_tile + A_cum * carry  is ONE 97x97 @ 97x512 matmul:
    lhsT rows 0..95 hold the triangular coefficient matrix M'^T
    (M'[i,j] = p_j * prod_{k=j+1..i} a_k), row 96 holds the carry
    coefficients A_cum[i] = prod_{k=tile_start..i} a_k; the rhs holds the
    96 concept rows plus the running carry (previous tile's last row) in
    row 96.  Engine partition ranges must start at base 0/32/64/96, so
    both the carry row of the rhs and the carry OUTPUT row must sit at
    partition 96: each tile gets a 97th lhsT column that replicates the
    96th (via an extra scan column with a=1, D=0), so the matmul writes
    y[last chunk of tile] to PSUM row 96, which the next tile's rhs copy
    may legally read.
  - All tiles' coefficient matrices are built on-device by a single
    tensor_tensor_scan over a [97, 97*ntiles] layout (state = a*state+D):
    scan column 97t+i maps to chunk 96t+i for i<96 (column 97t+96 is the
    replica column).  The per-tile restart is encoded by zeroing `a` at
    tile starts, the diagonal injection D[j, 97t+j] = p_eff[96t+j]
    produces M'^T and the row-96 injection D[96, 97t] = a[96t] produces
    A_cum.
  - Chunk states are duplicated to their 2 tokens during the output DMA.
"""

import math

import numpy as np

import concourse.bacc as bacc
import concourse.mybir as mybir
import concourse.tile as tile
from concourse.bass_utils import run_bass_kernel_spmd

F32 = mybir.dt.float32
L_TOK = 16384
L_CHK = 8192
HIDDEN = 4096
NCORES = 8
T = 96  # chunks per tile (partition 96 of each matmul carries the running state)

_cache: dict = {}

# set by test harnesses to capture a hardware profile of the SPMD run
_PROFILE = False
_last_result = None


def _build(Lc: int, Hc: int, dup: int, use_f32r: bool = True):
    """Build the per-core Bass program for Lc chunks x Hc hidden columns.

    The output DRAM tensor is [dup * Lc, Hc]; chunk c is written to rows
    dup*c .. dup*c+dup-1 (token plug-back for uniform chunks of `dup` tokens).
    """
    ntiles = math.ceil(Lc / T)
    S = T + 1  # scan columns per tile (96 chunks + 1 replica column)
    Ls = S * ntiles
    nseg = 512  # a_bc build segment (PSUM bank width in fp32)

    F32R = mybir.dt.float32r
    MMDT = F32R if use_f32r else F32
    nc = bacc.Bacc("TRN2", target_bir_lowering=False, debug=False)
    concept_d = nc.dram_tensor("concept_s", [Lc, Hc], MMDT, kind="ExternalInput")
    amask_d = nc.dram_tensor("a_masked", [1, Ls], F32, kind="ExternalInput")
    arow_d = nc.dram_tensor("arow", [1, Ls], F32, kind="ExternalInput")
    pt_d = nc.dram_tensor("pT", [128, ntiles], F32, kind="ExternalInput")
    ident_d = nc.dram_tensor("ident", [128, 128], F32, kind="ExternalInput")
    ones_d = nc.dram_tensor("ones_r", [1, 128], F32, kind="ExternalInput")
    zeros_d = nc.dram_tensor("zeros_t", [T, Hc], MMDT, kind="ExternalInput")
    out_d = nc.dram_tensor("out", [dup * Lc, Hc], F32, kind="ExternalOutput")
    ov = out_d.rearrange("(c two) h -> c two h", two=dup)

    with tile.TileContext(nc) as tc:
        with (
            tc.tile_pool(name="const", bufs=1) as cpool,
            tc.tile_pool(name="rhs", bufs=3) as rhspool,
            tc.tile_pool(name="ysb", bufs=2) as ypool,
            tc.tile_pool(name="psum", bufs=8, space="PSUM") as psumpool,
        ):
            ident = cpool.tile([128, 128], F32)
            ones_sb = cpool.tile([1, 128], F32)
            a_sb = cpool.tile([1, Ls], F32)
            pt_sb = cpool.tile([128, ntiles], F32)
            mt = cpool.tile([128, Ls], MMDT, tag="mt")
            a_bc = cpool.tile([128, Ls], F32, tag="a_bc")
            dmat = cpool.tile([128, Ls], F32, tag="dmat")

            nc.sync.dma_start(ident[:], ident_d[:])
            nc.sync.dma_start(ones_sb[:], ones_d[:])
            nc.sync.dma_start(a_sb[:], amask_d[:])
            nc.sync.dma_start(pt_sb[:], pt_d[:])
            # a_bc[p, c] = a_masked[c] for all p (ones (x) a outer products)
            for q in range(0, Ls, nseg):
                w = min(nseg, Ls - q)
                psb = psumpool.tile([128, nseg], F32, tag="ps")
                nc.tensor.matmul(
                    psb[:, 0:w], ones_sb[0:1, :], a_sb[0:1, q : q + w],
                    start=True, stop=True,
                )
                nc.vector.tensor_copy(a_bc[0:S, q : q + w], psb[0:S, 0:w])
            # row 96 of D: carry-coefficient injections at tile starts
            nc.sync.dma_start(dmat[96:97, :], arow_d[:])

            # D rows 0..95: diagonal p_eff injections (identity * p column);
            # identity column 96 zeroes the replica column of each block.
            for t in range(ntiles):
                nc.vector.tensor_scalar(
                    dmat[0:T, S * t : S * t + S],
                    ident[0:T, 0:S],
                    pt_sb[0:T, t : t + 1],
                    None,
                    mybir.AluOpType.mult,
                )

            # the big scan: MT[:, c] = a_bc[:, c] * MT[:, c-1] + D[:, c]
            seg = 2048
            for s in range(0, Ls, seg):
                w = min(seg, Ls - s)
                init = 0.0 if s == 0 else mt[0:S, s - 1 : s]
                nc.vector.tensor_tensor_scan(
                    mt[0:S, s : s + w],
                    a_bc[0:S, s : s + w],
                    dmat[0:S, s : s + w],
                    init,
                    mybir.AluOpType.mult,
                    mybir.AluOpType.add,
                )

            # main pipeline over chunk tiles, in groups of G tiles: one
            # batched input DMA and one batched output DMA per group.  The
            # carry recurrence is split into NCH independent hidden-column
            # chains (chain h's carry copy runs on its own engine) so the
            # per-tile serial latency overlaps across chains.  ybig holds
            # each y row duplicated dup times so every SBUF partition row is
            # one contiguous (dup*Hc) run matching the contiguous DRAM
            # region: the DMA write packets become 4KB instead of 2KB.
            G = 4
            NCH = 2
            Hch = Hc // NCH
            def _copy(eng, out, in_):
                if eng is nc.scalar:
                    eng.copy(out, in_)
                else:
                    eng.tensor_copy(out, in_)

            carry_eng = [nc.scalar, nc.scalar]
            nfull = Lc // T  # tiles with the full T chunks
            ngrp = nfull // G
            # grouped DRAM views: chunk row c = G*T*g + T*j + r
            if ngrp > 0:
                cv = concept_d[0 : ngrp * G * T, :].rearrange(
                    "(g j r) h -> g r j h", j=G, r=T
                )
                ovg = out_d[0 : dup * ngrp * G * T, :].rearrange(
                    "(g j r two) h -> g r j (two h)", j=G, r=T, two=dup
                )

            prev_ps = [None] * NCH
            for g in range(ngrp):
                btile = rhspool.tile([128, G * Hc], MMDT, tag="rhs")
                nc.scalar.dma_start(
                    btile[0:T, :].rearrange("r (j h) -> r j h", j=G), cv[g]
                )
                ybig = ypool.tile([128, G * dup * Hc], F32, tag="ysb")
                for j in range(G):
                    t = G * g + j
                    for h in range(NCH):
                        cols = slice(j * Hc + h * Hch, j * Hc + (h + 1) * Hch)
                        if t == 0:
                            nc.sync.dma_start(
                                btile[T : T + 1, cols], zeros_d[0:1, 0:Hch]
                            )
                        else:
                            _copy(
                                carry_eng[h],
                                btile[T : T + 1, cols],
                                prev_ps[h][T : T + 1, :],
                            )
                        ps = psumpool.tile([128, Hch], F32, tag="ps")
                        nc.tensor.matmul(
                            ps[0 : T + 1, :],
                            mt[0:S, S * t : S * t + S],
                            btile[0:S, cols],
                            start=True, stop=True,
                        )
                        prev_ps[h] = ps
                        ycols0 = slice(
                            j * dup * Hc + h * Hch, j * dup * Hc + (h + 1) * Hch
                        )
                        nc.vector.tensor_copy(ybig[0:T, ycols0], ps[0:T, :])
                        for r in range(1, dup):
                            ycols = slice(
                                (j * dup + r) * Hc + h * Hch,
                                (j * dup + r) * Hc + (h + 1) * Hch,
                            )
                            nc.gpsimd.tensor_copy(
                                ybig[0:T, ycols], ybig[0:T, ycols0]
                            )
                nc.sync.dma_start(
                    ovg[g],
                    ybig[0:T, :].rearrange("r (j x) -> r j x", j=G),
                )

            # leftover tiles (tail that doesn't fill a group)
            ovt = out_d.rearrange("(c two) h -> c (two h)", two=dup)
            for t in range(G * ngrp, ntiles):
                c0 = T * t
                w = min(T, Lc - c0)
                m = w + 1 if w == T else w  # full tiles emit the carry row
                rhs = rhspool.tile([128, G * Hc], MMDT, tag="rhs")
                if w < T:
                    # partial tile: zero the pad rows (their lhsT columns
                    # are all-zero, but operands must be initialized)
                    nc.sync.dma_start(rhs[0:T, 0:Hc], zeros_d[:])
                nc.scalar.dma_start(rhs[0:w, 0:Hc], concept_d[c0 : c0 + w, :])
                y_sb = ypool.tile([128, G * dup * Hc], F32, tag="ysb")
                for h in range(NCH):
                    cols = slice(h * Hch, (h + 1) * Hch)
                    if t == 0:
                        nc.sync.dma_start(
                            rhs[T : T + 1, cols], zeros_d[0:1, 0:Hch]
                        )
                    else:
                        _copy(
                            carry_eng[h],
                            rhs[T : T + 1, cols],
                            prev_ps[h][T : T + 1, :],
                        )
                    ps = psumpool.tile([128, Hch], F32, tag="ps")
                    nc.tensor.matmul(
                        ps[0:m, :],
                        mt[0:S, S * t : S * t + m],
                        rhs[0:S, cols],
                        start=True, stop=True,
                    )
                    prev_ps[h] = ps
                    ycols0 = slice(h * Hch, (h + 1) * Hch)
                    nc.vector.tensor_copy(y_sb[0:w, ycols0], ps[0:w, :])
                    for r in range(1, dup):
                        ycols = slice(r * Hc + h * Hch, r * Hc + (h + 1) * Hch)
                        nc.gpsimd.tensor_copy(y_sb[0:w, ycols], y_sb[0:w, ycols0])
                nc.sync.dma_start(ovt[c0 : c0 + w, :], y_sb[0:w, 0 : dup * Hc])

    nc.compile()
    return nc


def _get_program(Lc: int, Hc: int, dup: int):
    key = (Lc, Hc, dup)
    if key not in _cache:
        _cache[key] = _build(Lc, Hc, dup)
    return _cache[key]


def _host_prep(concept, selected_probs, boundary_mask):
    """Derive the per-chunk scan coefficients (replicated across cores)."""
    Lc = concept.shape[1]
    mask = np.asarray(boundary_mask[0])
    probs = np.asarray(selected_probs[0, :, 0], dtype=np.float32)

    idx = np.nonzero(mask)[0]
    if len(idx) < Lc:
        idx = np.pad(idx, (0, Lc - len(idx)))
    idx = idx[:Lc]
    p = probs[idx].astype(np.float32)
    a = (1.0 - p).astype(np.float32)
    a[0] = 0.0
    p_eff = p.copy()
    p_eff[0] = 1.0

    ntiles = math.ceil(Lc / T)
    S = T + 1
    Ls = S * ntiles
    # scan-space coefficients: column S*t+i <-> chunk T*t+i (i < T);
    # column S*t+T is the replica column (a=1, injections=0)
    a_scan = np.zeros(Ls, np.float32)
    arow = np.zeros(Ls, np.float32)
    pt = np.zeros((128, ntiles), np.float32)
    for t in range(ntiles):
        w = min(T, Lc - T * t)
        a_scan[S * t : S * t + w] = a[T * t : T * t + w]
        a_scan[S * t] = 0.0
        a_scan[S * t + T] = 1.0
        arow[S * t] = a[T * t]
        pt[:w, t] = p_eff[T * t : T * t + w]
    return a_scan[None, :], arow[None, :], pt, mask, idx


def kernel(concept, selected_probs, boundary_mask):
    concept = np.asarray(concept, dtype=np.float32)
    selected_probs = np.asarray(selected_probs, dtype=np.float32)
    boundary_mask = np.asarray(boundary_mask)

    Lc = concept.shape[1]
    L = boundary_mask.shape[1]
    H = concept.shape[2]
    Hc = H // NCORES

    a_masked, arow, pt, mask, idx = _host_prep(
        concept, selected_probs, boundary_mask
    )

    # fast path: uniform chunks of `dup` tokens (the setup_inputs pattern)
    dup = L // Lc
    expect = np.zeros(L, dtype=bool)
    expect[0::dup] = True
    uniform = dup * Lc == L and bool(np.array_equal(mask, expect))

    nc = _get_program(Lc, Hc, dup if uniform else 1)

    ident = np.eye(128, dtype=np.float32)
    ones_r = np.ones((1, 128), np.float32)
    zeros_t = np.zeros((T, Hc), np.float32)
    in_maps = []
    for k in range(NCORES):
        in_maps.append(
            {
                "concept_s": np.ascontiguousarray(
                    concept[0, :, k * Hc : (k + 1) * Hc]
                ),
                "a_masked": a_masked,
                "arow": arow,
                "pT": pt,
                "ident": ident,
                "ones_r": ones_r,
                "zeros_t": zeros_t,
            }
        )

    res = run_bass_kernel_spmd(
        nc, in_maps, list(range(NCORES)), trace=_PROFILE
    )
    global _last_result
    _last_result = res

    rows = dup * Lc if uniform else Lc
    full = np.empty((rows, H), np.float32)
    for k in range(NCORES):
        full[:, k * Hc : (k + 1) * Hc] = res.results[k]["out"]

    if uniform:
        return full[None]

    # general fallback: device computed the chunk states; plug back on host
    merge = full
    plug = np.cumsum(mask.astype(np.int64)) - 1
    plug = np.clip(plug, 0, Lc - 1)
    return merge[plug][None]
